# revision 1
# baseline (speedup 1.0000x reference)
"""Correlation kernel (FlowNet-style, W-displacement only) for Trainium2.

out[b, j, h, w] = mean_c f1[b,c,h,w] * f2pad[b,c,h,w+j],  j in [0, 81), pad=40.

Sharding: data-parallel over batch B=8 across 8 cores (1 batch elem/core).

Per-core pipeline (per h row):
  1. 3 matmuls (contraction over C=128 on partitions) produce Gram tiles
     G^T[w, u] = sum_c f1[c, w0+w] * f2p[c, w0+u] in PSUM.
  2. DVE/ACT copy PSUM -> SBUF.
  3. Band extraction: SBUF diagonal APs are illegal (partition steps must be
     partition-aligned), so bounce through DRAM: dump G^T tiles densely to a
     DRAM scratch, read back with a diagonal DRAM-side AP (flat, legal) so
     partition p holds out[p-th w, j=0..80].
  4. 3 PE transposes (identity matmul) -> PSUM tile [81, 320] (j on partitions).
  5. ACT copy (x 1/128) -> SBUF staging; chunk-batched contiguous DMA to DRAM.
"""

import numpy as np
from contextlib import ExitStack

B, C, H, W = 8, 128, 96, 320
D = 40
J = 2 * D + 1  # 81
WP = W + 2 * D  # 400
N_CORES = 8

HCHUNK = 16
NCHUNK = H // HCHUNK
# w-block starts; all matmuls padded to uniform M=128 (last block reads 64
# slack columns of garbage that the transpose never consumes)
WB = [0, 128, 256]
GN = 208  # matmul free dim / per-block width in gsb (= 128 + 2*D)
SLACK = 64


def _build(h_total=H):
    import concourse.bass as bass
    import concourse.tile as tile
    from concourse import bacc, mybir
    from concourse.masks import make_identity

    dt = mybir.dt.float32
    nc = bacc.Bacc(
        "TRN2",
        target_bir_lowering=False,
        debug=False,
        enable_asserts=False,
        num_devices=N_CORES,
    )
    f1 = nc.dram_tensor("f1", [C, h_total, W], dt, kind="ExternalInput").ap()
    f2 = nc.dram_tensor("f2", [C, h_total, W], dt, kind="ExternalInput").ap()
    out = nc.dram_tensor("out", [J, h_total, W], dt, kind="ExternalOutput").ap()

    nchunk = h_total // HCHUNK

    with tile.TileContext(nc) as tc, ExitStack() as ctx:
        const_pool = ctx.enter_context(tc.tile_pool(name="const", bufs=1))
        scr_pool = ctx.enter_context(tc.tile_pool(name="scr", bufs=8, space="DRAM"))
        f1_pool = ctx.enter_context(tc.tile_pool(name="f1p", bufs=2))
        f2_pool = ctx.enter_context(tc.tile_pool(name="f2p", bufs=2))
        g_pool = ctx.enter_context(tc.tile_pool(name="gsb", bufs=4))
        ral_pool = ctx.enter_context(tc.tile_pool(name="ral", bufs=4))
        ost_pool = ctx.enter_context(tc.tile_pool(name="ost", bufs=2))
        psg_pool = ctx.enter_context(tc.tile_pool(name="psg", bufs=6, space="PSUM"))
        pst_pool = ctx.enter_context(tc.tile_pool(name="pst", bufs=2, space="PSUM"))

        ident = const_pool.tile([128, 128], dt)
        make_identity(nc, ident[:])

        for ci in range(nchunk):
            h0 = ci * HCHUNK
            f1s = f1_pool.tile([C, HCHUNK * W + SLACK], dt)
            nc.vector.memset(f1s[:, HCHUNK * W :], 0.0)
            nc.sync.dma_start(f1s[:, 0 : HCHUNK * W], f1[:, h0 : h0 + HCHUNK, :])
            f2ps = f2_pool.tile([C, HCHUNK * WP + SLACK], dt)
            f2v = f2ps[:, 0 : HCHUNK * WP].rearrange("p (h w) -> p h w", h=HCHUNK)
            # zero the pad columns + slack, then land the data between them
            nc.vector.memset(f2v[:, :, 0:D], 0.0)
            nc.vector.memset(f2v[:, :, W + D : WP], 0.0)
            nc.vector.memset(f2ps[:, HCHUNK * WP :], 0.0)
            nc.sync.dma_start(f2v[:, :, D : W + D], f2[:, h0 : h0 + HCHUNK, :])

            ost = ost_pool.tile([J, HCHUNK * W], dt)
            for h in range(HCHUNK):
                base1 = h * W
                base2 = h * WP
                gsb = g_pool.tile([C, 3 * GN], dt)
                for bi, w0 in enumerate(WB):
                    pg = psg_pool.tile([128, GN], dt, tag="pg")
                    nc.tensor.matmul(
                        pg[:],
                        lhsT=f1s[:, base1 + w0 : base1 + w0 + 128],
                        rhs=f2ps[:, base2 + w0 : base2 + w0 + GN],
                        start=True,
                        stop=True,
                    )
                    if bi < 2:
                        nc.vector.tensor_copy(gsb[:, bi * GN : (bi + 1) * GN], pg[:])
                    else:
                        nc.scalar.copy(gsb[:, bi * GN : (bi + 1) * GN], pg[:])

                # band extraction via DRAM bounce: dense dump, diagonal read-back
                scr = scr_pool.tile([C, 3 * GN], dt)
                nc.scalar.dma_start(scr[:], gsb[:])
                ss = scr[:]
                diag_src = bass.AP(
                    ss.tensor, ss.offset, [[ss.ap[0][0] + 1, 128], [GN, 3], [1, J]]
                )
                ral = ral_pool.tile([C, 3 * J], dt)
                rs = ral[:]
                diag_dst = bass.AP(
                    rs.tensor, rs.offset, [[rs.ap[0][0], 128], [J, 3], [1, J]]
                )
                nc.sync.dma_start(diag_dst, diag_src)

                pt = pst_pool.tile([J, W], dt, tag="pt")
                for bi, w0 in enumerate(WB):
                    kp = min(128, W - w0)
                    nc.tensor.transpose(
                        pt[0:J, w0 : w0 + kp],
                        ral[0:kp, bi * J : bi * J + J],
                        ident[0:kp, 0:kp],
                    )
                nc.scalar.mul(ost[:, base1 : base1 + W], pt[:], 1.0 / C)

            nc.sync.dma_start(out[:, h0 : h0 + HCHUNK, :], ost[:])

    nc.finalize()
    return nc


def _run(nc, in_maps, **kwargs):
    from concourse.bass_utils import run_bass_kernel_spmd

    return run_bass_kernel_spmd(nc, in_maps, core_ids=list(range(N_CORES)), **kwargs)


def kernel(f1: np.ndarray, f2: np.ndarray, **run_kwargs) -> np.ndarray:
    assert f1.shape == (B, C, H, W) and f2.shape == (B, C, H, W)
    nc = _build()
    in_maps = [
        {
            "f1": np.ascontiguousarray(f1[i], dtype=np.float32),
            "f2": np.ascontiguousarray(f2[i], dtype=np.float32),
        }
        for i in range(N_CORES)
    ]
    res = _run(nc, in_maps, **run_kwargs)
    out = np.stack([r["out"] for r in res.results], axis=0)
    if run_kwargs:
        kernel.last_results = res
    return out



# revision 6
# speedup vs baseline: 1.2193x; 1.2193x over previous
"""Correlation kernel (FlowNet-style, W-displacement only) for Trainium2.

out[b, j, h, w] = mean_c f1[b,c,h,w] * f2pad[b,c,h,w+j],  j in [0, 81), pad=40.

Sharding: data-parallel over batch B=8 across 8 cores (1 batch elem/core).

v2 design (all bf16 on the wire; ~33MB DMA/core vs 84MB fp32 baseline):
  - Host casts f1*(1/C) and f2 to bf16; device output is bf16, host upcasts.
  - Per h row: 3 matmuls (C=128 contraction on partitions) -> Gram tiles
    G[w, u] in PSUM (fp32), blocks M=128/128/64, rhs windows N=208/208/144.
    Two consecutive rows share each PSUM tile so evictions batch 2 rows.
  - Compaction: PSUM->SBUF bf16 copies write 64-partition-group windows of
    144 els (band j in [0,81) lives at [q, q+81), q = p mod 64), so the
    DRAM bounce moves 432 (lo) / 288 (hi partitions, 2 blocks) els/row.
  - Band extraction via DRAM bounce (SBUF diagonal APs are illegal): dump
    4 rows per DMA (split lo/hi partitions), read back with per-64-group
    diagonal APs (stride pitch+1) landing ral[p, bi*81+j] as 81-el bursts.
  - 3 PE transposes per row (identity matmul, bf16 1 cyc/col) -> [81, 2W]
    bf16 PSUM, single ACT copy -> bf16 staging, chunked contiguous DMA out.
  - Software pipelining: pack k's readback/transpose work is emitted after
    pack k+2's matmuls so the bounce round-trip hides behind PE work.
"""

import numpy as np
from contextlib import ExitStack

B, C, H, W = 8, 128, 96, 320
D = 40
J = 2 * D + 1  # 81
WP = W + 2 * D  # 400
N_CORES = 8

HCHUNK = 16
NCHUNK = H // HCHUNK
ROWPACK = 4  # rows per scratch dump DMA
GRP = 64  # compaction group size (q = p mod GRP)
SLOT = GRP + 2 * D  # 144 window els per block slot
LO_ROW = 3 * SLOT  # 432 els/row, partitions 0..63 (3 block slots)
HI_ROW = 2 * SLOT  # 288 els/row, partitions 64..127 (2 block slots)
LO_PACK = ROWPACK * LO_ROW  # 1728
HI_PACK = ROWPACK * HI_ROW  # 1152
# blocks: (w0, M rows of lhsT, N rhs cols)
WBLK = [(0, 128, 208), (128, 128, 208), (256, 64, 144)]
PPC = HCHUNK // ROWPACK  # packs per chunk


def _build(h_total=H):
    import concourse.bass as bass
    import concourse.tile as tile
    from concourse import bacc, mybir
    from concourse.masks import make_identity

    bf = mybir.dt.bfloat16
    f32 = mybir.dt.float32
    nc = bacc.Bacc(
        "TRN2",
        target_bir_lowering=False,
        debug=False,
        enable_asserts=False,
        num_devices=N_CORES,
    )
    f1 = nc.dram_tensor("f1", [C, h_total, W], bf, kind="ExternalInput").ap()
    f2 = nc.dram_tensor("f2", [C, h_total, W], bf, kind="ExternalInput").ap()
    out = nc.dram_tensor("out", [J, h_total, W], bf, kind="ExternalOutput").ap()

    nchunk = h_total // HCHUNK
    npack = nchunk * PPC

    with tile.TileContext(nc) as tc, ExitStack() as ctx:
        const_pool = ctx.enter_context(tc.tile_pool(name="const", bufs=1))
        scra_pool = ctx.enter_context(tc.tile_pool(name="scra", bufs=8, space="DRAM"))
        scrb_pool = ctx.enter_context(tc.tile_pool(name="scrb", bufs=8, space="DRAM"))
        f1_pool = ctx.enter_context(tc.tile_pool(name="f1p", bufs=2))
        f2d_pool = ctx.enter_context(tc.tile_pool(name="f2d", bufs=2))
        f2p_pool = ctx.enter_context(tc.tile_pool(name="f2p", bufs=2))
        g_pool = ctx.enter_context(tc.tile_pool(name="gsb", bufs=3))
        ral_pool = ctx.enter_context(tc.tile_pool(name="ral", bufs=8))
        ost_pool = ctx.enter_context(tc.tile_pool(name="ost", bufs=2))
        pg0_pool = ctx.enter_context(tc.tile_pool(name="pg0", bufs=2, space="PSUM"))
        pg1_pool = ctx.enter_context(tc.tile_pool(name="pg1", bufs=2, space="PSUM"))
        pg2_pool = ctx.enter_context(tc.tile_pool(name="pg2", bufs=2, space="PSUM"))
        pst_pool = ctx.enter_context(tc.tile_pool(name="pst", bufs=2, space="PSUM"))

        ident = const_pool.tile([128, 128], bf)
        make_identity(nc, ident[:])

        chunk_tiles = {}  # ci -> (f1s, f2ps, ost)
        pack_state = {}  # k -> (scrA, scrB, ci, hstart)

        def front(k):
            ci, hp = divmod(k, PPC)
            if hp == 0:
                h0 = ci * HCHUNK
                f1s = f1_pool.tile([C, HCHUNK * W], bf)
                nc.sync.dma_start(f1s[:], f1[:, h0 : h0 + HCHUNK, :])
                f2d = f2d_pool.tile([C, HCHUNK * W], bf)
                nc.sync.dma_start(f2d[:], f2[:, h0 : h0 + HCHUNK, :])
                # repack dense rows into per-row padded layout [h, 400]
                f2ps = f2p_pool.tile([C, HCHUNK * WP], bf)
                f2v = f2ps[:].rearrange("p (h w) -> p h w", h=HCHUNK)
                nc.vector.memset(f2v[:, :, 0:D], 0.0)
                nc.vector.memset(f2v[:, :, W + D : WP], 0.0)
                nc.gpsimd.tensor_copy(
                    f2v[:, :, D : W + D],
                    f2d[:].rearrange("p (h w) -> p h w", h=HCHUNK),
                )
                ost = ost_pool.tile([J, HCHUNK * W], bf)
                chunk_tiles[ci] = (f1s, f2ps, ost)
            f1s, f2ps, ost = chunk_tiles[ci]
            hstart = hp * ROWPACK  # row within chunk

            gsb = g_pool.tile([C, LO_PACK], bf)
            for rp in range(ROWPACK // 2):
                pgs = [
                    pg0_pool.tile([128, 416], f32, tag="a", name="pga"),
                    pg1_pool.tile([128, 416], f32, tag="b", name="pgb"),
                    pg2_pool.tile([64, 288], f32, tag="c", name="pgc"),
                ]
                for r2 in range(2):
                    h = hstart + rp * 2 + r2
                    base1 = h * W
                    base2 = h * WP
                    for bi, (w0, m, n) in enumerate(WBLK):
                        coff = r2 * n
                        nc.tensor.matmul(
                            pgs[bi][0:m, coff : coff + n],
                            lhsT=f1s[:, base1 + w0 : base1 + w0 + m],
                            rhs=f2ps[:, base2 + w0 : base2 + w0 + n],
                            start=True,
                            stop=True,
                        )
                # compaction copies (2 rows each): band of partition p lives
                # at window [q, q+81), q = p mod 64, of its 144-el slot
                lor = rp * 2 * LO_ROW
                hir = rp * 2 * HI_ROW
                dlo = gsb[0:64, lor : lor + 2 * LO_ROW].rearrange(
                    "p (r n) -> p r n", r=2
                )
                dhi = gsb[64:128, hir : hir + 2 * HI_ROW].rearrange(
                    "p (r n) -> p r n", r=2
                )
                s0 = pgs[0][:, :].rearrange("p (r n) -> p r n", r=2)
                s1 = pgs[1][:, :].rearrange("p (r n) -> p r n", r=2)
                s2 = pgs[2][:, :].rearrange("p (r n) -> p r n", r=2)
                nc.vector.tensor_copy(dlo[:, :, 0:SLOT], s0[0:64, :, 0:SLOT])
                nc.vector.tensor_copy(dhi[:, :, 0:SLOT], s0[64:128, :, 64:208])
                nc.vector.tensor_copy(dhi[:, :, SLOT : 2 * SLOT], s1[64:128, :, 64:208])
                nc.scalar.copy(dlo[:, :, SLOT : 2 * SLOT], s1[0:64, :, 0:SLOT])
                nc.scalar.copy(dlo[:, :, 2 * SLOT : 3 * SLOT], s2[0:64, :, :])

            scrA = scra_pool.tile([64, LO_PACK], bf)
            nc.scalar.dma_start(scrA[:], gsb[0:64, :])
            scrB = scrb_pool.tile([64, HI_PACK], bf)
            nc.scalar.dma_start(scrB[:], gsb[64:128, 0:HI_PACK])
            pack_state[k] = (scrA, scrB, ci, hstart)

        def back(k):
            scrA, scrB, ci, hstart = pack_state.pop(k)
            _, _, ost = chunk_tiles[ci]
            sa = scrA[:]
            sb = scrB[:]
            for rp in range(ROWPACK // 2):
                pst = pst_pool.tile([J, 2 * W], bf, tag="t")
                for r2 in range(2):
                    r = rp * 2 + r2
                    ral = ral_pool.tile([C, 3 * J], bf)
                    srcA = bass.AP(
                        sa.tensor,
                        sa.offset + r * LO_ROW,
                        [[LO_PACK + 1, 64], [SLOT, 3], [1, J]],
                    )
                    nc.sync.dma_start(ral[0:64, 0 : 3 * J], srcA)
                    srcB = bass.AP(
                        sb.tensor,
                        sb.offset + r * HI_ROW,
                        [[HI_PACK + 1, 64], [SLOT, 2], [1, J]],
                    )
                    nc.sync.dma_start(ral[64:128, 0 : 2 * J], srcB)
                    for bi, (w0, m, n) in enumerate(WBLK):
                        nc.tensor.transpose(
                            pst[0:J, r2 * W + w0 : r2 * W + w0 + m],
                            ral[0:m, bi * J : bi * J + J],
                            ident[0:m, 0:m],
                        )
                b1 = (hstart + rp * 2) * W
                nc.scalar.copy(ost[:, b1 : b1 + 2 * W], pst[:])
            if hstart // ROWPACK == PPC - 1:
                h0 = ci * HCHUNK
                nc.sync.dma_start(out[:, h0 : h0 + HCHUNK, :], ost[:])

        SKEW = 2
        for k in range(npack):
            front(k)
            if k >= SKEW:
                back(k - SKEW)
        for k in range(npack - SKEW, npack):
            back(k)

    nc.finalize()
    return nc


def _run(nc, in_maps, **kwargs):
    from concourse.bass_utils import run_bass_kernel_spmd

    return run_bass_kernel_spmd(nc, in_maps, core_ids=list(range(N_CORES)), **kwargs)


def kernel(f1: np.ndarray, f2: np.ndarray, **run_kwargs) -> np.ndarray:
    import ml_dtypes

    assert f1.shape == (B, C, H, W) and f2.shape == (B, C, H, W)
    bf16 = ml_dtypes.bfloat16
    scale = np.float32(1.0 / C)
    nc = _build()
    in_maps = [
        {
            "f1": np.ascontiguousarray(
                (np.asarray(f1[i], dtype=np.float32) * scale).astype(bf16)
            ),
            "f2": np.ascontiguousarray(np.asarray(f2[i], dtype=np.float32).astype(bf16)),
        }
        for i in range(N_CORES)
    ]
    res = _run(nc, in_maps, **run_kwargs)
    out = np.stack([np.asarray(r["out"], dtype=np.float32) for r in res.results], axis=0)
    if run_kwargs:
        kernel.last_results = res
    return out


# revision 10
# speedup vs baseline: 1.4592x; 1.1967x over previous
"""Correlation kernel (FlowNet-style, W-displacement only) for Trainium2.

out[b, j, h, w] = mean_c f1[b,c,h,w] * f2pad[b,c,h,w+j],  j in [0, 81), pad=40.

Sharding: data-parallel over batch B=8 across 8 cores (1 batch elem/core).

v2 design (all bf16 on the wire; ~33MB DMA/core vs 84MB fp32 baseline):
  - Host casts f1*(1/C) and f2 to bf16; device output is bf16, host upcasts.
  - Per h row: 3 matmuls (C=128 contraction on partitions) -> Gram tiles
    G[w, u] in PSUM (fp32), blocks M=128/128/64, rhs windows N=208/208/144.
    Two consecutive rows share each PSUM tile so evictions batch 2 rows.
  - Compaction: PSUM->SBUF bf16 copies write 64-partition-group windows of
    144 els (band j in [0,81) lives at [q, q+81), q = p mod 64), so the
    DRAM bounce moves 432 (lo) / 288 (hi partitions, 2 blocks) els/row.
  - Band extraction via DRAM bounce (SBUF diagonal APs are illegal): dump
    4 rows per DMA (split lo/hi partitions), read back with per-64-group
    diagonal APs (stride pitch+1) landing ral[p, bi*81+j] as 81-el bursts.
  - 3 PE transposes per row (identity matmul, bf16 1 cyc/col) -> [81, 2W]
    bf16 PSUM, single ACT copy -> bf16 staging, chunked contiguous DMA out.
  - Software pipelining: pack k's readback/transpose work is emitted after
    pack k+2's matmuls so the bounce round-trip hides behind PE work.
"""

import numpy as np
from contextlib import ExitStack

B, C, H, W = 8, 128, 96, 320
D = 40
J = 2 * D + 1  # 81
WP = W + 2 * D  # 400
N_CORES = 8

HCHUNK = 16
NCHUNK = H // HCHUNK
ROWPACK = 4  # rows per scratch dump DMA
GRP = 64  # compaction group size (q = p mod GRP)
SLOT = GRP + 2 * D  # 144 window els per block slot
LO_ROW = 3 * SLOT  # 432 els/row, partitions 0..63 (3 block slots)
HI_ROW = 2 * SLOT  # 288 els/row, partitions 64..127 (2 block slots)
LO_PACK = ROWPACK * LO_ROW  # 1728
HI_PACK = ROWPACK * HI_ROW  # 1152
# blocks: (w0, M rows of lhsT, N rhs cols)
WBLK = [(0, 128, 208), (128, 128, 208), (256, 64, 144)]
PPC = HCHUNK // ROWPACK  # packs per chunk


def _build(h_total=H):
    import concourse.bass as bass
    import concourse.tile as tile
    from concourse import bacc, mybir
    from concourse.masks import make_identity

    bf = mybir.dt.bfloat16
    f32 = mybir.dt.float32
    nc = bacc.Bacc(
        "TRN2",
        target_bir_lowering=False,
        debug=False,
        enable_asserts=False,
        num_devices=N_CORES,
    )
    f1 = nc.dram_tensor("f1", [C, h_total, W], bf, kind="ExternalInput").ap()
    f2 = nc.dram_tensor("f2", [C, h_total, W], bf, kind="ExternalInput").ap()
    out = nc.dram_tensor("out", [J, h_total, W], bf, kind="ExternalOutput").ap()

    nchunk = h_total // HCHUNK
    npack = nchunk * PPC

    with tile.TileContext(nc) as tc, ExitStack() as ctx:
        const_pool = ctx.enter_context(tc.tile_pool(name="const", bufs=1))
        scra_pool = ctx.enter_context(tc.tile_pool(name="scra", bufs=8, space="DRAM"))
        scrb_pool = ctx.enter_context(tc.tile_pool(name="scrb", bufs=8, space="DRAM"))
        f1_pool = ctx.enter_context(tc.tile_pool(name="f1p", bufs=2))
        f2p_pool = ctx.enter_context(tc.tile_pool(name="f2p", bufs=2))
        g_pool = ctx.enter_context(tc.tile_pool(name="gsb", bufs=3))
        ral_pool = ctx.enter_context(tc.tile_pool(name="ral", bufs=8))
        ost_pool = ctx.enter_context(tc.tile_pool(name="ost", bufs=3))
        pg0_pool = ctx.enter_context(tc.tile_pool(name="pg0", bufs=2, space="PSUM"))
        pg1_pool = ctx.enter_context(tc.tile_pool(name="pg1", bufs=2, space="PSUM"))
        pg2_pool = ctx.enter_context(tc.tile_pool(name="pg2", bufs=2, space="PSUM"))
        pst_pool = ctx.enter_context(tc.tile_pool(name="pst", bufs=2, space="PSUM"))

        ident = const_pool.tile([128, 128], bf)
        make_identity(nc, ident[:])

        chunk_tiles = {}  # ci -> (f1s, f2ps, ost)
        pack_state = {}  # k -> (scrA, scrB, ci, hstart)

        def front(k):
            ci, hp = divmod(k, PPC)
            if hp == 0:
                h0 = ci * HCHUNK
                f1s = f1_pool.tile([C, HCHUNK * W], bf)
                nc.scalar.dma_start(f1s[:], f1[:, h0 : h0 + HCHUNK, :])
                # strided load straight into per-row padded layout [h, 400]
                f2ps = f2p_pool.tile([C, HCHUNK * WP], bf)
                f2v = f2ps[:].rearrange("p (h w) -> p h w", h=HCHUNK)
                nc.vector.memset(f2v[:, :, 0:D], 0.0)
                nc.vector.memset(f2v[:, :, W + D : WP], 0.0)
                nc.scalar.dma_start(f2v[:, :, D : W + D], f2[:, h0 : h0 + HCHUNK, :])
                ost = ost_pool.tile([J, HCHUNK * W], bf)
                chunk_tiles[ci] = (f1s, f2ps, ost)
            f1s, f2ps, ost = chunk_tiles[ci]
            hstart = hp * ROWPACK  # row within chunk

            gsb = g_pool.tile([C, LO_PACK], bf)
            for rp in range(ROWPACK // 2):
                pgs = [
                    pg0_pool.tile([128, 416], f32, tag="a", name="pga"),
                    pg1_pool.tile([128, 416], f32, tag="b", name="pgb"),
                    pg2_pool.tile([64, 288], f32, tag="c", name="pgc"),
                ]
                for r2 in range(2):
                    h = hstart + rp * 2 + r2
                    base1 = h * W
                    base2 = h * WP
                    for bi, (w0, m, n) in enumerate(WBLK):
                        coff = r2 * n
                        nc.tensor.matmul(
                            pgs[bi][0:m, coff : coff + n],
                            lhsT=f1s[:, base1 + w0 : base1 + w0 + m],
                            rhs=f2ps[:, base2 + w0 : base2 + w0 + n],
                            start=True,
                            stop=True,
                        )
                # compaction copies (2 rows each): band of partition p lives
                # at window [q, q+81), q = p mod 64, of its 144-el slot
                lor = rp * 2 * LO_ROW
                hir = rp * 2 * HI_ROW
                dlo = gsb[0:64, lor : lor + 2 * LO_ROW].rearrange(
                    "p (r n) -> p r n", r=2
                )
                dhi = gsb[64:128, hir : hir + 2 * HI_ROW].rearrange(
                    "p (r n) -> p r n", r=2
                )
                s0 = pgs[0][:, :].rearrange("p (r n) -> p r n", r=2)
                s1 = pgs[1][:, :].rearrange("p (r n) -> p r n", r=2)
                s2 = pgs[2][:, :].rearrange("p (r n) -> p r n", r=2)
                nc.vector.tensor_copy(dlo[:, :, 0:SLOT], s0[0:64, :, 0:SLOT])
                nc.vector.tensor_copy(dhi[:, :, 0:SLOT], s0[64:128, :, 64:208])
                nc.vector.tensor_copy(dhi[:, :, SLOT : 2 * SLOT], s1[64:128, :, 64:208])
                nc.vector.tensor_copy(dlo[:, :, 2 * SLOT : 3 * SLOT], s2[0:64, :, :])
                nc.scalar.copy(dlo[:, :, SLOT : 2 * SLOT], s1[0:64, :, 0:SLOT])

            scrA = scra_pool.tile([64, LO_PACK], bf)
            nc.scalar.dma_start(scrA[:], gsb[0:64, :])
            scrB = scrb_pool.tile([64, HI_PACK], bf)
            nc.scalar.dma_start(scrB[:], gsb[64:128, 0:HI_PACK])
            pack_state[k] = (scrA, scrB, ci, hstart)

        def back(k):
            scrA, scrB, ci, hstart = pack_state.pop(k)
            _, _, ost = chunk_tiles[ci]
            sa = scrA[:]
            sb = scrB[:]
            for rp in range(ROWPACK // 2):
                pst = pst_pool.tile([J, 2 * W], bf, tag="t")
                for r2 in range(2):
                    r = rp * 2 + r2
                    ral = ral_pool.tile([C, 3 * J], bf)
                    srcA = bass.AP(
                        sa.tensor,
                        sa.offset + r * LO_ROW,
                        [[LO_PACK + 1, 64], [SLOT, 3], [1, J]],
                    )
                    nc.sync.dma_start(ral[0:64, 0 : 3 * J], srcA)
                    srcB = bass.AP(
                        sb.tensor,
                        sb.offset + r * HI_ROW,
                        [[HI_PACK + 1, 64], [SLOT, 2], [1, J]],
                    )
                    nc.sync.dma_start(ral[64:128, 0 : 2 * J], srcB)
                    for bi, (w0, m, n) in enumerate(WBLK):
                        nc.tensor.transpose(
                            pst[0:J, r2 * W + w0 : r2 * W + w0 + m],
                            ral[0:m, bi * J : bi * J + J],
                            ident[0:m, 0:m],
                        )
                b1 = (hstart + rp * 2) * W
                nc.scalar.copy(ost[:, b1 : b1 + 2 * W], pst[:])
            if hstart // ROWPACK == PPC - 1:
                h0 = ci * HCHUNK
                nc.scalar.dma_start(out[:, h0 : h0 + HCHUNK, :], ost[:])

        SKEW = 4
        for k in range(npack):
            front(k)
            if k >= SKEW:
                back(k - SKEW)
        for k in range(npack - SKEW, npack):
            back(k)

    nc.finalize()
    return nc


def _run(nc, in_maps, **kwargs):
    from concourse.bass_utils import run_bass_kernel_spmd

    return run_bass_kernel_spmd(nc, in_maps, core_ids=list(range(N_CORES)), **kwargs)


def kernel(f1: np.ndarray, f2: np.ndarray, **run_kwargs) -> np.ndarray:
    import ml_dtypes

    assert f1.shape == (B, C, H, W) and f2.shape == (B, C, H, W)
    bf16 = ml_dtypes.bfloat16
    scale = np.float32(1.0 / C)
    nc = _build()
    in_maps = [
        {
            "f1": np.ascontiguousarray(
                (np.asarray(f1[i], dtype=np.float32) * scale).astype(bf16)
            ),
            "f2": np.ascontiguousarray(np.asarray(f2[i], dtype=np.float32).astype(bf16)),
        }
        for i in range(N_CORES)
    ]
    res = _run(nc, in_maps, **run_kwargs)
    out = np.stack([np.asarray(r["out"], dtype=np.float32) for r in res.results], axis=0)
    if run_kwargs:
        kernel.last_results = res
    return out


# revision 12
# speedup vs baseline: 1.5987x; 1.0956x over previous
"""Correlation kernel (FlowNet-style, W-displacement only) for Trainium2.

out[b, j, h, w] = mean_c f1[b,c,h,w] * f2pad[b,c,h,w+j],  j in [0, 81), pad=40.

Sharding: data-parallel over batch B=8 across 8 cores (1 batch elem/core).

v2 design (all bf16 on the wire; ~33MB DMA/core vs 84MB fp32 baseline):
  - Host casts f1*(1/C) and f2 to bf16; device output is bf16, host upcasts.
  - Per h row: 3 matmuls (C=128 contraction on partitions) -> Gram tiles
    G[w, u] in PSUM (fp32), blocks M=128/128/64, rhs windows N=208/208/144.
    Two consecutive rows share each PSUM tile so evictions batch 2 rows.
  - Compaction: PSUM->SBUF bf16 copies write 64-partition-group windows of
    144 els (band j in [0,81) lives at [q, q+81), q = p mod 64), so the
    DRAM bounce moves 432 (lo) / 288 (hi partitions, 2 blocks) els/row.
  - Band extraction via DRAM bounce (SBUF diagonal APs are illegal): dump
    4 rows per DMA (split lo/hi partitions), read back with per-64-group
    diagonal APs (stride pitch+1) landing ral[p, bi*81+j] as 81-el bursts.
  - 3 PE transposes per row (identity matmul, bf16 1 cyc/col) -> [81, 2W]
    bf16 PSUM, single ACT copy -> bf16 staging, chunked contiguous DMA out.
  - Software pipelining: pack k's readback/transpose work is emitted after
    pack k+2's matmuls so the bounce round-trip hides behind PE work.
"""

import numpy as np
from contextlib import ExitStack

B, C, H, W = 8, 128, 96, 320
D = 40
J = 2 * D + 1  # 81
WP = W + 2 * D  # 400
N_CORES = 8

HCHUNK = 16
NCHUNK = H // HCHUNK
ROWPACK = 4  # rows per scratch dump DMA
SLOT = 208  # els per block slot (dense: band of partition p at [p, p+81))
ROWELS = 3 * SLOT  # 624 els per row per partition
PACKELS = ROWPACK * ROWELS  # 2496
# blocks: (w0, M rows of lhsT, N rhs cols)
WBLK = [(0, 128, 208), (128, 128, 208), (256, 64, 144)]
PPC = HCHUNK // ROWPACK  # packs per chunk


def _build(h_total=H):
    import concourse.bass as bass
    import concourse.tile as tile
    from concourse import bacc, mybir
    from concourse.masks import make_identity

    bf = mybir.dt.bfloat16
    f32 = mybir.dt.float32
    nc = bacc.Bacc(
        "TRN2",
        target_bir_lowering=False,
        debug=False,
        enable_asserts=False,
        num_devices=N_CORES,
    )
    f1 = nc.dram_tensor("f1", [C, h_total, W], bf, kind="ExternalInput").ap()
    f2 = nc.dram_tensor("f2", [C, h_total, W], bf, kind="ExternalInput").ap()
    out = nc.dram_tensor("out", [J, h_total, W], bf, kind="ExternalOutput").ap()

    nchunk = h_total // HCHUNK
    npack = nchunk * PPC

    with tile.TileContext(nc) as tc, ExitStack() as ctx:
        const_pool = ctx.enter_context(tc.tile_pool(name="const", bufs=1))
        scr_pool = ctx.enter_context(tc.tile_pool(name="scr", bufs=8, space="DRAM"))
        f1_pool = ctx.enter_context(tc.tile_pool(name="f1p", bufs=2))
        f2p_pool = ctx.enter_context(tc.tile_pool(name="f2p", bufs=2))
        g_pool = ctx.enter_context(tc.tile_pool(name="gsb", bufs=3))
        ral_pool = ctx.enter_context(tc.tile_pool(name="ral", bufs=8))
        ost_pool = ctx.enter_context(tc.tile_pool(name="ost", bufs=3))
        pg0_pool = ctx.enter_context(tc.tile_pool(name="pg0", bufs=2, space="PSUM"))
        pg1_pool = ctx.enter_context(tc.tile_pool(name="pg1", bufs=2, space="PSUM"))
        pg2_pool = ctx.enter_context(tc.tile_pool(name="pg2", bufs=2, space="PSUM"))
        pst_pool = ctx.enter_context(tc.tile_pool(name="pst", bufs=2, space="PSUM"))

        ident = const_pool.tile([128, 128], bf)
        make_identity(nc, ident[:])

        chunk_tiles = {}  # ci -> (f1s, f2ps, ost)
        pack_state = {}  # k -> (scrA, scrB, ci, hstart)

        def front(k):
            ci, hp = divmod(k, PPC)
            if hp == 0:
                h0 = ci * HCHUNK
                f1s = f1_pool.tile([C, HCHUNK * W], bf)
                nc.scalar.dma_start(f1s[:], f1[:, h0 : h0 + HCHUNK, :])
                # strided load straight into per-row padded layout [h, 400]
                f2ps = f2p_pool.tile([C, HCHUNK * WP], bf)
                f2v = f2ps[:].rearrange("p (h w) -> p h w", h=HCHUNK)
                nc.vector.memset(f2v[:, :, 0:D], 0.0)
                nc.vector.memset(f2v[:, :, W + D : WP], 0.0)
                nc.scalar.dma_start(f2v[:, :, D : W + D], f2[:, h0 : h0 + HCHUNK, :])
                ost = ost_pool.tile([J, HCHUNK * W], bf)
                chunk_tiles[ci] = (f1s, f2ps, ost)
            f1s, f2ps, ost = chunk_tiles[ci]
            hstart = hp * ROWPACK  # row within chunk

            gsb = g_pool.tile([C, PACKELS], bf)
            for rp in range(ROWPACK // 2):
                pgs = [
                    pg0_pool.tile([128, 416], f32, tag="a", name="pga"),
                    pg1_pool.tile([128, 416], f32, tag="b", name="pgb"),
                    pg2_pool.tile([64, 288], f32, tag="c", name="pgc"),
                ]
                for r2 in range(2):
                    h = hstart + rp * 2 + r2
                    base1 = h * W
                    base2 = h * WP
                    for bi, (w0, m, n) in enumerate(WBLK):
                        coff = r2 * n
                        nc.tensor.matmul(
                            pgs[bi][0:m, coff : coff + n],
                            lhsT=f1s[:, base1 + w0 : base1 + w0 + m],
                            rhs=f2ps[:, base2 + w0 : base2 + w0 + n],
                            start=True,
                            stop=True,
                        )
                # eviction copies (2 rows each) into dense 208-el slots:
                # band of partition p lives at [p, p+81) of its block slot
                base = rp * 2 * ROWELS
                dst = gsb[:, base : base + 2 * ROWELS].rearrange(
                    "p (r n) -> p r n", r=2
                )
                s0 = pgs[0][:, :].rearrange("p (r n) -> p r n", r=2)
                s1 = pgs[1][:, :].rearrange("p (r n) -> p r n", r=2)
                s2 = pgs[2][:, :].rearrange("p (r n) -> p r n", r=2)
                nc.vector.tensor_copy(dst[0:128, :, 0:208], s0)
                nc.vector.tensor_copy(dst[0:64, :, 2 * SLOT : 2 * SLOT + 144], s2)
                nc.scalar.copy(dst[0:128, :, SLOT : SLOT + 208], s1)

            scr = scr_pool.tile([C, PACKELS], bf)
            nc.scalar.dma_start(scr[:], gsb[:])
            pack_state[k] = (scr, ci, hstart)

        def back(k):
            scr, ci, hstart = pack_state.pop(k)
            _, _, ost = chunk_tiles[ci]
            ss = scr[:]
            for rp in range(ROWPACK // 2):
                pst = pst_pool.tile([J, 2 * W], bf, tag="t")
                for r2 in range(2):
                    r = rp * 2 + r2
                    ral = ral_pool.tile([C, 3 * J], bf)
                    src = bass.AP(
                        ss.tensor,
                        ss.offset + r * ROWELS,
                        [[PACKELS + 1, 128], [SLOT, 3], [1, J]],
                    )
                    nc.sync.dma_start(ral[:, 0 : 3 * J], src)
                    for bi, (w0, m, n) in enumerate(WBLK):
                        nc.tensor.transpose(
                            pst[0:J, r2 * W + w0 : r2 * W + w0 + m],
                            ral[0:m, bi * J : bi * J + J],
                            ident[0:m, 0:m],
                        )
                b1 = (hstart + rp * 2) * W
                nc.scalar.copy(ost[:, b1 : b1 + 2 * W], pst[:])
            if hstart // ROWPACK == PPC - 1:
                h0 = ci * HCHUNK
                nc.scalar.dma_start(out[:, h0 : h0 + HCHUNK, :], ost[:])

        SKEW = 4
        for k in range(npack):
            front(k)
            if k >= SKEW:
                back(k - SKEW)
        for k in range(npack - SKEW, npack):
            back(k)

    nc.finalize()
    return nc


def _run(nc, in_maps, **kwargs):
    from concourse.bass_utils import run_bass_kernel_spmd

    return run_bass_kernel_spmd(nc, in_maps, core_ids=list(range(N_CORES)), **kwargs)


def kernel(f1: np.ndarray, f2: np.ndarray, **run_kwargs) -> np.ndarray:
    import ml_dtypes

    assert f1.shape == (B, C, H, W) and f2.shape == (B, C, H, W)
    bf16 = ml_dtypes.bfloat16
    scale = np.float32(1.0 / C)
    nc = _build()
    in_maps = [
        {
            "f1": np.ascontiguousarray(
                (np.asarray(f1[i], dtype=np.float32) * scale).astype(bf16)
            ),
            "f2": np.ascontiguousarray(np.asarray(f2[i], dtype=np.float32).astype(bf16)),
        }
        for i in range(N_CORES)
    ]
    res = _run(nc, in_maps, **run_kwargs)
    out = np.stack([np.asarray(r["out"], dtype=np.float32) for r in res.results], axis=0)
    if run_kwargs:
        kernel.last_results = res
    return out


# revision 13
# speedup vs baseline: 1.7467x; 1.0926x over previous
"""Correlation kernel (FlowNet-style, W-displacement only) for Trainium2.

out[b, j, h, w] = mean_c f1[b,c,h,w] * f2pad[b,c,h,w+j],  j in [0, 81), pad=40.

Sharding: data-parallel over batch B=8 across 8 cores (1 batch elem/core).

v2 design (all bf16 on the wire; ~33MB DMA/core vs 84MB fp32 baseline):
  - Host casts f1*(1/C) and f2 to bf16; device output is bf16, host upcasts.
  - Per h row: 3 matmuls (C=128 contraction on partitions) -> Gram tiles
    G[w, u] in PSUM (fp32), blocks M=128/128/64, rhs windows N=208/208/144.
    Two consecutive rows share each PSUM tile so evictions batch 2 rows.
  - Compaction: PSUM->SBUF bf16 copies write 64-partition-group windows of
    144 els (band j in [0,81) lives at [q, q+81), q = p mod 64), so the
    DRAM bounce moves 432 (lo) / 288 (hi partitions, 2 blocks) els/row.
  - Band extraction via DRAM bounce (SBUF diagonal APs are illegal): dump
    4 rows per DMA (split lo/hi partitions), read back with per-64-group
    diagonal APs (stride pitch+1) landing ral[p, bi*81+j] as 81-el bursts.
  - 3 PE transposes per row (identity matmul, bf16 1 cyc/col) -> [81, 2W]
    bf16 PSUM, single ACT copy -> bf16 staging, chunked contiguous DMA out.
  - Software pipelining: pack k's readback/transpose work is emitted after
    pack k+2's matmuls so the bounce round-trip hides behind PE work.
"""

import numpy as np
from contextlib import ExitStack

B, C, H, W = 8, 128, 96, 320
D = 40
J = 2 * D + 1  # 81
WP = W + 2 * D  # 400
N_CORES = 8

HCHUNK = 16
NCHUNK = H // HCHUNK
ROWPACK = 4  # rows per scratch dump DMA
SLOT = 208  # els per block slot (dense: band of partition p at [p, p+81))
ROWELS = 3 * SLOT  # 624 els per row per partition
PACKELS = ROWPACK * ROWELS  # 2496
# blocks: (w0, M rows of lhsT, N rhs cols)
WBLK = [(0, 128, 208), (128, 128, 208), (256, 64, 144)]
PPC = HCHUNK // ROWPACK  # packs per chunk


def _build(h_total=H):
    import concourse.bass as bass
    import concourse.tile as tile
    from concourse import bacc, mybir
    from concourse.masks import make_identity

    bf = mybir.dt.bfloat16
    f32 = mybir.dt.float32
    nc = bacc.Bacc(
        "TRN2",
        target_bir_lowering=False,
        debug=False,
        enable_asserts=False,
        num_devices=N_CORES,
    )
    f1 = nc.dram_tensor("f1", [C, h_total, W], bf, kind="ExternalInput").ap()
    f2 = nc.dram_tensor("f2", [C, h_total, W], bf, kind="ExternalInput").ap()
    out = nc.dram_tensor("out", [J, h_total, W], bf, kind="ExternalOutput").ap()

    nchunk = h_total // HCHUNK
    npack = nchunk * PPC

    with tile.TileContext(nc) as tc, ExitStack() as ctx:
        const_pool = ctx.enter_context(tc.tile_pool(name="const", bufs=1))
        scr_pool = ctx.enter_context(tc.tile_pool(name="scr", bufs=8, space="DRAM"))
        f1_pool = ctx.enter_context(tc.tile_pool(name="f1p", bufs=2))
        f2p_pool = ctx.enter_context(tc.tile_pool(name="f2p", bufs=2))
        g_pool = ctx.enter_context(tc.tile_pool(name="gsb", bufs=3))
        ral_pool = ctx.enter_context(tc.tile_pool(name="ral", bufs=8))
        ost_pool = ctx.enter_context(tc.tile_pool(name="ost", bufs=3))
        pg0_pool = ctx.enter_context(tc.tile_pool(name="pg0", bufs=2, space="PSUM"))
        pg1_pool = ctx.enter_context(tc.tile_pool(name="pg1", bufs=2, space="PSUM"))
        pg2_pool = ctx.enter_context(tc.tile_pool(name="pg2", bufs=2, space="PSUM"))
        pst_pool = ctx.enter_context(tc.tile_pool(name="pst", bufs=2, space="PSUM"))

        ident = const_pool.tile([128, 128], bf)
        make_identity(nc, ident[:])

        chunk_tiles = {}  # ci -> (f1s, f2ps, ost)
        pack_state = {}  # k -> (scrA, scrB, ci, hstart)

        def front(k):
            ci, hp = divmod(k, PPC)
            if hp == 0:
                h0 = ci * HCHUNK
                f1s = f1_pool.tile([C, HCHUNK * W], bf)
                nc.sync.dma_start(f1s[:], f1[:, h0 : h0 + HCHUNK, :])
                # strided load straight into per-row padded layout [h, 400]
                f2ps = f2p_pool.tile([C, HCHUNK * WP], bf)
                f2v = f2ps[:].rearrange("p (h w) -> p h w", h=HCHUNK)
                nc.vector.memset(f2v[:, :, 0:D], 0.0)
                nc.vector.memset(f2v[:, :, W + D : WP], 0.0)
                nc.sync.dma_start(f2v[:, :, D : W + D], f2[:, h0 : h0 + HCHUNK, :])
                ost = ost_pool.tile([J, HCHUNK * W], bf)
                chunk_tiles[ci] = (f1s, f2ps, ost)
            f1s, f2ps, ost = chunk_tiles[ci]
            hstart = hp * ROWPACK  # row within chunk

            gsb = g_pool.tile([C, PACKELS], bf)
            for rp in range(ROWPACK // 2):
                pgs = [
                    pg0_pool.tile([128, 416], f32, tag="a", name="pga"),
                    pg1_pool.tile([128, 416], f32, tag="b", name="pgb"),
                    pg2_pool.tile([64, 288], f32, tag="c", name="pgc"),
                ]
                for r2 in range(2):
                    h = hstart + rp * 2 + r2
                    base1 = h * W
                    base2 = h * WP
                    for bi, (w0, m, n) in enumerate(WBLK):
                        coff = r2 * n
                        nc.tensor.matmul(
                            pgs[bi][0:m, coff : coff + n],
                            lhsT=f1s[:, base1 + w0 : base1 + w0 + m],
                            rhs=f2ps[:, base2 + w0 : base2 + w0 + n],
                            start=True,
                            stop=True,
                        )
                # eviction copies (2 rows each) into dense 208-el slots:
                # band of partition p lives at [p, p+81) of its block slot
                base = rp * 2 * ROWELS
                dst = gsb[:, base : base + 2 * ROWELS].rearrange(
                    "p (r n) -> p r n", r=2
                )
                s0 = pgs[0][:, :].rearrange("p (r n) -> p r n", r=2)
                s1 = pgs[1][:, :].rearrange("p (r n) -> p r n", r=2)
                s2 = pgs[2][:, :].rearrange("p (r n) -> p r n", r=2)
                nc.vector.tensor_copy(dst[0:128, :, 0:208], s0)
                nc.vector.tensor_copy(dst[0:64, :, 2 * SLOT : 2 * SLOT + 144], s2)
                nc.scalar.copy(dst[0:128, :, SLOT : SLOT + 208], s1)

            scr = scr_pool.tile([C, PACKELS], bf)
            nc.scalar.dma_start(scr[:], gsb[:])
            pack_state[k] = (scr, ci, hstart)

        def back(k):
            scr, ci, hstart = pack_state.pop(k)
            _, _, ost = chunk_tiles[ci]
            ss = scr[:]
            for rp in range(ROWPACK // 2):
                pst = pst_pool.tile([J, 2 * W], bf, tag="t")
                for r2 in range(2):
                    r = rp * 2 + r2
                    ral = ral_pool.tile([C, 3 * J], bf)
                    src = bass.AP(
                        ss.tensor,
                        ss.offset + r * ROWELS,
                        [[PACKELS + 1, 128], [SLOT, 3], [1, J]],
                    )
                    nc.sync.dma_start(ral[:, 0 : 3 * J], src)
                    for bi, (w0, m, n) in enumerate(WBLK):
                        nc.tensor.transpose(
                            pst[0:J, r2 * W + w0 : r2 * W + w0 + m],
                            ral[0:m, bi * J : bi * J + J],
                            ident[0:m, 0:m],
                        )
                b1 = (hstart + rp * 2) * W
                nc.vector.tensor_copy(ost[:, b1 : b1 + 2 * W], pst[:])
            if hstart // ROWPACK == PPC - 1:
                h0 = ci * HCHUNK
                nc.scalar.dma_start(out[:, h0 : h0 + HCHUNK, :], ost[:])

        SKEW = 6
        for k in range(npack):
            front(k)
            if k >= SKEW:
                back(k - SKEW)
        for k in range(npack - SKEW, npack):
            back(k)

    nc.finalize()
    return nc


def _run(nc, in_maps, **kwargs):
    from concourse.bass_utils import run_bass_kernel_spmd

    return run_bass_kernel_spmd(nc, in_maps, core_ids=list(range(N_CORES)), **kwargs)


def kernel(f1: np.ndarray, f2: np.ndarray, **run_kwargs) -> np.ndarray:
    import ml_dtypes

    assert f1.shape == (B, C, H, W) and f2.shape == (B, C, H, W)
    bf16 = ml_dtypes.bfloat16
    scale = np.float32(1.0 / C)
    nc = _build()
    in_maps = [
        {
            "f1": np.ascontiguousarray(
                (np.asarray(f1[i], dtype=np.float32) * scale).astype(bf16)
            ),
            "f2": np.ascontiguousarray(np.asarray(f2[i], dtype=np.float32).astype(bf16)),
        }
        for i in range(N_CORES)
    ]
    res = _run(nc, in_maps, **run_kwargs)
    out = np.stack([np.asarray(r["out"], dtype=np.float32) for r in res.results], axis=0)
    if run_kwargs:
        kernel.last_results = res
    return out


# revision 18
# speedup vs baseline: 1.9730x; 1.1296x over previous
"""Correlation kernel (FlowNet-style, W-displacement only) for Trainium2.

out[b, j, h, w] = mean_c f1[b,c,h,w] * f2pad[b,c,h,w+j],  j in [0, 81), pad=40.

Sharding: data-parallel over batch B=8 across 8 cores (1 batch elem/core).

v2 design (all bf16 on the wire; ~33MB DMA/core vs 84MB fp32 baseline):
  - Host casts f1*(1/C) and f2 to bf16; device output is bf16, host upcasts.
  - Per h row: 3 matmuls (C=128 contraction on partitions) -> Gram tiles
    G[w, u] in PSUM (fp32), blocks M=128/128/64, rhs windows N=208/208/144.
    Two consecutive rows share each PSUM tile so evictions batch 2 rows.
  - Compaction: PSUM->SBUF bf16 copies write 64-partition-group windows of
    144 els (band j in [0,81) lives at [q, q+81), q = p mod 64), so the
    DRAM bounce moves 432 (lo) / 288 (hi partitions, 2 blocks) els/row.
  - Band extraction via DRAM bounce (SBUF diagonal APs are illegal): dump
    4 rows per DMA (split lo/hi partitions), read back with per-64-group
    diagonal APs (stride pitch+1) landing ral[p, bi*81+j] as 81-el bursts.
  - 3 PE transposes per row (identity matmul, bf16 1 cyc/col) -> [81, 2W]
    bf16 PSUM, single ACT copy -> bf16 staging, chunked contiguous DMA out.
  - Software pipelining: pack k's readback/transpose work is emitted after
    pack k+2's matmuls so the bounce round-trip hides behind PE work.
"""

import numpy as np
from contextlib import ExitStack

B, C, H, W = 8, 128, 96, 320
D = 40
J = 2 * D + 1  # 81
WP = W + 2 * D  # 400
N_CORES = 8

HCHUNK = 16
NCHUNK = H // HCHUNK
ROWPACK = 4  # rows per scratch dump DMA
SLOT = 188  # els per block slot (dense: band of partition p at [p, p+81))
ROWELS = 3 * SLOT  # 564 els per row per partition
PACKELS = ROWPACK * ROWELS  # 2256
# blocks: (w0, M rows of lhsT, N rhs cols); even widths keep PSUM 4B-aligned
WBLK = [(0, 108, 188), (108, 108, 188), (216, 104, 184)]
PPC = HCHUNK // ROWPACK  # packs per chunk


def _build(h_total=H):
    import concourse.bass as bass
    import concourse.tile as tile
    from concourse import bacc, mybir
    from concourse.masks import make_identity

    bf = mybir.dt.bfloat16
    f32 = mybir.dt.float32
    nc = bacc.Bacc(
        "TRN2",
        target_bir_lowering=False,
        debug=False,
        enable_asserts=False,
        num_devices=N_CORES,
    )
    f1 = nc.dram_tensor("f1", [C, h_total, W], bf, kind="ExternalInput").ap()
    f2 = nc.dram_tensor("f2", [C, h_total, W], bf, kind="ExternalInput").ap()
    out = nc.dram_tensor("out", [J, h_total, W], bf, kind="ExternalOutput").ap()

    nchunk = h_total // HCHUNK
    npack = nchunk * PPC

    with tile.TileContext(nc) as tc, ExitStack() as ctx:
        const_pool = ctx.enter_context(tc.tile_pool(name="const", bufs=1))
        scr_pool = ctx.enter_context(tc.tile_pool(name="scr", bufs=8, space="DRAM"))
        f1_pool = ctx.enter_context(tc.tile_pool(name="f1p", bufs=2))
        f2p_pool = ctx.enter_context(tc.tile_pool(name="f2p", bufs=2))
        g_pool = ctx.enter_context(tc.tile_pool(name="gsb", bufs=3))
        ral_pool = ctx.enter_context(tc.tile_pool(name="ral", bufs=8))
        ost_pool = ctx.enter_context(tc.tile_pool(name="ost", bufs=3))
        pg0_pool = ctx.enter_context(tc.tile_pool(name="pg0", bufs=2, space="PSUM"))
        pg1_pool = ctx.enter_context(tc.tile_pool(name="pg1", bufs=2, space="PSUM"))
        pg2_pool = ctx.enter_context(tc.tile_pool(name="pg2", bufs=2, space="PSUM"))
        pst_pool = ctx.enter_context(tc.tile_pool(name="pst", bufs=2, space="PSUM"))

        ident = const_pool.tile([128, 128], bf)
        make_identity(nc, ident[:])

        chunk_tiles = {}  # ci -> (f1s, f2ps, ost)
        pack_state = {}  # k -> (scrA, scrB, ci, hstart)

        def front(k):
            ci, hp = divmod(k, PPC)
            if hp == 0:
                h0 = ci * HCHUNK
                f1s = f1_pool.tile([C, HCHUNK * W], bf)
                nc.sync.dma_start(f1s[:], f1[:, h0 : h0 + HCHUNK, :])
                # strided load straight into per-row padded layout [h, 400]
                f2ps = f2p_pool.tile([C, HCHUNK * WP], bf)
                f2v = f2ps[:].rearrange("p (h w) -> p h w", h=HCHUNK)
                nc.vector.memset(f2v[:, :, 0:D], 0.0)
                nc.vector.memset(f2v[:, :, W + D : WP], 0.0)
                nc.sync.dma_start(f2v[:, :, D : W + D], f2[:, h0 : h0 + HCHUNK, :])
                ost = ost_pool.tile([J, HCHUNK * W], bf)
                chunk_tiles[ci] = (f1s, f2ps, ost)
            f1s, f2ps, ost = chunk_tiles[ci]
            hstart = hp * ROWPACK  # row within chunk

            gsb = g_pool.tile([C, PACKELS], bf)
            for rp in range(ROWPACK // 2):
                pgs = [
                    pg0_pool.tile([128, 376], f32, tag="a", name="pga"),
                    pg1_pool.tile([128, 376], f32, tag="b", name="pgb"),
                    pg2_pool.tile([104, 368], f32, tag="c", name="pgc"),
                ]
                for r2 in range(2):
                    h = hstart + rp * 2 + r2
                    base1 = h * W
                    base2 = h * WP
                    for bi, (w0, m, n) in enumerate(WBLK):
                        coff = r2 * n
                        nc.tensor.matmul(
                            pgs[bi][0:m, coff : coff + n],
                            lhsT=f1s[:, base1 + w0 : base1 + w0 + m],
                            rhs=f2ps[:, base2 + w0 : base2 + w0 + n],
                            start=True,
                            stop=True,
                        )
                # eviction copies (2 rows each) into dense 208-el slots:
                # band of partition p lives at [p, p+81) of its block slot
                base = rp * 2 * ROWELS
                dst = gsb[:, base : base + 2 * ROWELS].rearrange(
                    "p (r n) -> p r n", r=2
                )
                s0 = pgs[0][:, :].rearrange("p (r n) -> p r n", r=2)
                s1 = pgs[1][:, :].rearrange("p (r n) -> p r n", r=2)
                s2 = pgs[2][:, :].rearrange("p (r n) -> p r n", r=2)
                nc.vector.tensor_copy(dst[0:108, :, 0:188], s0[0:108])
                nc.vector.tensor_copy(dst[0:104, :, 2 * SLOT : 2 * SLOT + 184], s2[0:104])
                nc.scalar.copy(dst[0:108, :, SLOT : SLOT + 188], s1[0:108])

            scr = scr_pool.tile([C, PACKELS + 64], bf)
            nc.scalar.dma_start(scr[:, 0:PACKELS], gsb[:])
            pack_state[k] = (scr, ci, hstart)

        def back(k):
            scr, ci, hstart = pack_state.pop(k)
            _, _, ost = chunk_tiles[ci]
            ss = scr[:]
            # diagonal readback, 2 rows per DMA: (row, block) fold into one
            # uniform-stride dim because ROWELS == 3*SLOT
            ral = ral_pool.tile([C, ROWPACK * 3 * J], bf)
            for rr in range(ROWPACK // 2):
                rsrc = bass.AP(
                    ss.tensor,
                    ss.offset + rr * 2 * ROWELS,
                    [[PACKELS + 64 + 1, 128], [SLOT, 6], [1, J]],
                )
                nc.sync.dma_start(
                    ral[:, rr * 6 * J : (rr + 1) * 6 * J], rsrc
                )
            for rp in range(ROWPACK // 2):
                pst = pst_pool.tile([J, 2 * W], bf, tag="t")
                for r2 in range(2):
                    r = rp * 2 + r2
                    for bi, (w0, m, n) in enumerate(WBLK):
                        nc.tensor.transpose(
                            pst[0:J, r2 * W + w0 : r2 * W + w0 + m],
                            ral[0:m, (3 * r + bi) * J : (3 * r + bi) * J + J],
                            ident[0:m, 0:m],
                        )
                b1 = (hstart + rp * 2) * W
                nc.vector.tensor_copy(ost[:, b1 : b1 + 2 * W], pst[:])
            if hstart // ROWPACK == PPC - 1:
                h0 = ci * HCHUNK
                nc.scalar.dma_start(out[:, h0 : h0 + HCHUNK, :], ost[:])

        SKEW = 6
        for k in range(npack):
            front(k)
            if k >= SKEW:
                back(k - SKEW)
        for k in range(npack - SKEW, npack):
            back(k)

    nc.finalize()
    return nc


def _run(nc, in_maps, **kwargs):
    from concourse.bass_utils import run_bass_kernel_spmd

    return run_bass_kernel_spmd(nc, in_maps, core_ids=list(range(N_CORES)), **kwargs)


def kernel(f1: np.ndarray, f2: np.ndarray, **run_kwargs) -> np.ndarray:
    import ml_dtypes

    assert f1.shape == (B, C, H, W) and f2.shape == (B, C, H, W)
    bf16 = ml_dtypes.bfloat16
    scale = np.float32(1.0 / C)
    nc = _build()
    in_maps = [
        {
            "f1": np.ascontiguousarray(
                (np.asarray(f1[i], dtype=np.float32) * scale).astype(bf16)
            ),
            "f2": np.ascontiguousarray(np.asarray(f2[i], dtype=np.float32).astype(bf16)),
        }
        for i in range(N_CORES)
    ]
    res = _run(nc, in_maps, **run_kwargs)
    out = np.stack([np.asarray(r["out"], dtype=np.float32) for r in res.results], axis=0)
    if run_kwargs:
        kernel.last_results = res
    return out


# revision 19
# speedup vs baseline: 2.0205x; 1.0241x over previous
"""Correlation kernel (FlowNet-style, W-displacement only) for Trainium2.

out[b, j, h, w] = mean_c f1[b,c,h,w] * f2pad[b,c,h,w+j],  j in [0, 81), pad=40.

Sharding: data-parallel over batch B=8 across 8 cores (1 batch elem/core).

v2 design (all bf16 on the wire; ~33MB DMA/core vs 84MB fp32 baseline):
  - Host casts f1*(1/C) and f2 to bf16; device output is bf16, host upcasts.
  - Per h row: 3 matmuls (C=128 contraction on partitions) -> Gram tiles
    G[w, u] in PSUM (fp32), blocks M=128/128/64, rhs windows N=208/208/144.
    Two consecutive rows share each PSUM tile so evictions batch 2 rows.
  - Compaction: PSUM->SBUF bf16 copies write 64-partition-group windows of
    144 els (band j in [0,81) lives at [q, q+81), q = p mod 64), so the
    DRAM bounce moves 432 (lo) / 288 (hi partitions, 2 blocks) els/row.
  - Band extraction via DRAM bounce (SBUF diagonal APs are illegal): dump
    4 rows per DMA (split lo/hi partitions), read back with per-64-group
    diagonal APs (stride pitch+1) landing ral[p, bi*81+j] as 81-el bursts.
  - 3 PE transposes per row (identity matmul, bf16 1 cyc/col) -> [81, 2W]
    bf16 PSUM, single ACT copy -> bf16 staging, chunked contiguous DMA out.
  - Software pipelining: pack k's readback/transpose work is emitted after
    pack k+2's matmuls so the bounce round-trip hides behind PE work.
"""

import numpy as np
from contextlib import ExitStack

B, C, H, W = 8, 128, 96, 320
D = 40
J = 2 * D + 1  # 81
WP = W + 2 * D  # 400
N_CORES = 8

HCHUNK = 16
NCHUNK = H // HCHUNK
ROWPACK = 4  # rows per scratch dump DMA
SLOT = 188  # els per block slot (dense: band of partition p at [p, p+81))
ROWELS = 3 * SLOT  # 564 els per row per partition
PACKELS = ROWPACK * ROWELS  # 2256
# blocks: (w0, M rows of lhsT, N rhs cols); even widths keep PSUM 4B-aligned
WBLK = [(0, 108, 188), (108, 108, 188), (216, 104, 184)]
PPC = HCHUNK // ROWPACK  # packs per chunk


def _build(h_total=H):
    import concourse.bass as bass
    import concourse.tile as tile
    from concourse import bacc, mybir
    from concourse.masks import make_identity

    bf = mybir.dt.bfloat16
    f32 = mybir.dt.float32
    nc = bacc.Bacc(
        "TRN2",
        target_bir_lowering=False,
        debug=False,
        enable_asserts=False,
        num_devices=N_CORES,
    )
    f1 = nc.dram_tensor("f1", [C, h_total, W], bf, kind="ExternalInput").ap()
    f2 = nc.dram_tensor("f2", [C, h_total, W], bf, kind="ExternalInput").ap()
    out = nc.dram_tensor("out", [J, h_total, W], bf, kind="ExternalOutput").ap()

    nchunk = h_total // HCHUNK
    npack = nchunk * PPC

    with tile.TileContext(nc) as tc, ExitStack() as ctx:
        const_pool = ctx.enter_context(tc.tile_pool(name="const", bufs=1))
        scr_pool = ctx.enter_context(tc.tile_pool(name="scr", bufs=10, space="DRAM"))
        f1_pool = ctx.enter_context(tc.tile_pool(name="f1p", bufs=2))
        f2d_pool = ctx.enter_context(tc.tile_pool(name="f2d", bufs=2))
        f2p_pool = ctx.enter_context(tc.tile_pool(name="f2p", bufs=2))
        g_pool = ctx.enter_context(tc.tile_pool(name="gsb", bufs=3))
        ral_pool = ctx.enter_context(tc.tile_pool(name="ral", bufs=8))
        ost_pool = ctx.enter_context(tc.tile_pool(name="ost", bufs=3))
        pg0_pool = ctx.enter_context(tc.tile_pool(name="pg0", bufs=2, space="PSUM"))
        pg1_pool = ctx.enter_context(tc.tile_pool(name="pg1", bufs=2, space="PSUM"))
        pg2_pool = ctx.enter_context(tc.tile_pool(name="pg2", bufs=2, space="PSUM"))
        pst_pool = ctx.enter_context(tc.tile_pool(name="pst", bufs=2, space="PSUM"))

        ident = const_pool.tile([128, 128], bf)
        make_identity(nc, ident[:])

        chunk_tiles = {}  # ci -> (f1s, f2ps, ost)
        pack_state = {}  # k -> (scrA, scrB, ci, hstart)

        def front(k):
            ci, hp = divmod(k, PPC)
            if hp == 0:
                h0 = ci * HCHUNK
                f1s = f1_pool.tile([C, HCHUNK * W], bf)
                nc.sync.dma_start(f1s[:], f1[:, h0 : h0 + HCHUNK, :])
                # dense f2 load; ACT repacks into per-row padded layout [h, 400]
                f2d = f2d_pool.tile([C, HCHUNK * W], bf)
                nc.sync.dma_start(f2d[:], f2[:, h0 : h0 + HCHUNK, :])
                f2ps = f2p_pool.tile([C, HCHUNK * WP], bf)
                f2v = f2ps[:].rearrange("p (h w) -> p h w", h=HCHUNK)
                nc.vector.memset(f2v[:, :, 0:D], 0.0)
                nc.vector.memset(f2v[:, :, W + D : WP], 0.0)
                nc.scalar.copy(
                    f2v[:, :, D : W + D],
                    f2d[:].rearrange("p (h w) -> p h w", h=HCHUNK),
                )
                ost = ost_pool.tile([J, HCHUNK * W], bf)
                chunk_tiles[ci] = (f1s, f2ps, ost)
            f1s, f2ps, ost = chunk_tiles[ci]
            hstart = hp * ROWPACK  # row within chunk

            gsb = g_pool.tile([C, PACKELS], bf)
            for rp in range(ROWPACK // 2):
                pgs = [
                    pg0_pool.tile([128, 376], f32, tag="a", name="pga"),
                    pg1_pool.tile([128, 376], f32, tag="b", name="pgb"),
                    pg2_pool.tile([104, 368], f32, tag="c", name="pgc"),
                ]
                for r2 in range(2):
                    h = hstart + rp * 2 + r2
                    base1 = h * W
                    base2 = h * WP
                    for bi, (w0, m, n) in enumerate(WBLK):
                        coff = r2 * n
                        nc.tensor.matmul(
                            pgs[bi][0:m, coff : coff + n],
                            lhsT=f1s[:, base1 + w0 : base1 + w0 + m],
                            rhs=f2ps[:, base2 + w0 : base2 + w0 + n],
                            start=True,
                            stop=True,
                        )
                # eviction copies (2 rows each) into dense 208-el slots:
                # band of partition p lives at [p, p+81) of its block slot
                base = rp * 2 * ROWELS
                dst = gsb[:, base : base + 2 * ROWELS].rearrange(
                    "p (r n) -> p r n", r=2
                )
                s0 = pgs[0][:, :].rearrange("p (r n) -> p r n", r=2)
                s1 = pgs[1][:, :].rearrange("p (r n) -> p r n", r=2)
                s2 = pgs[2][:, :].rearrange("p (r n) -> p r n", r=2)
                nc.vector.tensor_copy(dst[0:108, :, 0:188], s0[0:108])
                nc.vector.tensor_copy(dst[0:108, :, SLOT : SLOT + 188], s1[0:108])
                nc.scalar.copy(dst[0:104, :, 2 * SLOT : 2 * SLOT + 184], s2[0:104])

            scr = scr_pool.tile([C, PACKELS + 64], bf)
            nc.scalar.dma_start(scr[:, 0:PACKELS], gsb[:])
            pack_state[k] = (scr, ci, hstart)

        def back(k):
            scr, ci, hstart = pack_state.pop(k)
            _, _, ost = chunk_tiles[ci]
            ss = scr[:]
            # diagonal readback, 2 rows per DMA: (row, block) fold into one
            # uniform-stride dim because ROWELS == 3*SLOT
            ral = ral_pool.tile([C, ROWPACK * 3 * J], bf)
            for rr in range(ROWPACK // 2):
                rsrc = bass.AP(
                    ss.tensor,
                    ss.offset + rr * 2 * ROWELS,
                    [[PACKELS + 64 + 1, 128], [SLOT, 6], [1, J]],
                )
                nc.sync.dma_start(
                    ral[:, rr * 6 * J : (rr + 1) * 6 * J], rsrc
                )
            for rp in range(ROWPACK // 2):
                pst = pst_pool.tile([J, 2 * W], bf, tag="t")
                for r2 in range(2):
                    r = rp * 2 + r2
                    for bi, (w0, m, n) in enumerate(WBLK):
                        nc.tensor.transpose(
                            pst[0:J, r2 * W + w0 : r2 * W + w0 + m],
                            ral[0:m, (3 * r + bi) * J : (3 * r + bi) * J + J],
                            ident[0:m, 0:m],
                        )
                b1 = (hstart + rp * 2) * W
                nc.scalar.copy(ost[:, b1 : b1 + 2 * W], pst[:])
            if hstart // ROWPACK == PPC - 1:
                h0 = ci * HCHUNK
                nc.scalar.dma_start(out[:, h0 : h0 + HCHUNK, :], ost[:])

        SKEW = 8
        for k in range(npack):
            front(k)
            if k >= SKEW:
                back(k - SKEW)
        for k in range(npack - SKEW, npack):
            back(k)

    nc.finalize()
    return nc


def _run(nc, in_maps, **kwargs):
    from concourse.bass_utils import run_bass_kernel_spmd

    return run_bass_kernel_spmd(nc, in_maps, core_ids=list(range(N_CORES)), **kwargs)


def kernel(f1: np.ndarray, f2: np.ndarray, **run_kwargs) -> np.ndarray:
    import ml_dtypes

    assert f1.shape == (B, C, H, W) and f2.shape == (B, C, H, W)
    bf16 = ml_dtypes.bfloat16
    scale = np.float32(1.0 / C)
    nc = _build()
    in_maps = [
        {
            "f1": np.ascontiguousarray(
                (np.asarray(f1[i], dtype=np.float32) * scale).astype(bf16)
            ),
            "f2": np.ascontiguousarray(np.asarray(f2[i], dtype=np.float32).astype(bf16)),
        }
        for i in range(N_CORES)
    ]
    res = _run(nc, in_maps, **run_kwargs)
    out = np.stack([np.asarray(r["out"], dtype=np.float32) for r in res.results], axis=0)
    if run_kwargs:
        kernel.last_results = res
    return out
